# revision 1
# baseline (speedup 1.0000x reference)
"""DenseSNN Trainium2 kernel: 4-layer LIF SNN, T=100 steps, B=128, D=H=2048, C=100.

Strategy
--------
The reference scans timesteps with all 4 layers inside the scan body, but the
dependency structure is feed-forward across layers: layer-l spikes at step t
depend only on layer-(l-1) spikes at steps <= t. So the computation unrolls into
per-layer phases:

    CUR1 = x @ W1 + b1          (batched over all T*B rows)
    S1   = LIF-scan_T(CUR1)     (elementwise in (B,H), sequential in T)
    CUR2 = S1 @ W2 + b2 ; S2 = LIF-scan(CUR2)
    CUR3 = S2 @ W3 + b3 ; S3 = LIF-scan(CUR3)
    CURo = S3 @ Wo + bo ; out = sum_t LIF-scan(CURo)

This turns the tiny per-step GEMMs into full-size GEMMs and makes pure
data-parallelism over batch (16 samples/core on 8 cores) communication-free.

On-chip layout is "transposed activations": [feature -> 16 chunks x 128
partitions, (t,b) -> free axis]. Weight-stationary matmuls (lhsT = W tile in
natural [D,H] layout) keep every tensor in this layout end to end; the host
pre-transposes x and re-assembles the output, so the device never transposes.

Matmuls run in bf16 (inputs cast on host) with fp32 PSUM accumulation; LIF
membrane state is fp32 on the vector engine. Spikes are exactly representable
in bf16. reset(t) == spike(t-1), which saves one compare per step.
"""

import numpy as np
import ml_dtypes

import concourse.bass as bass
import concourse.mybir as mybir
import concourse.tile as tile
from concourse import bacc
from concourse.bass_utils import run_bass_kernel_spmd

# Problem constants (hardcoded per contract)
T, B, D, H, C = 100, 128, 2048, 2048, 100
NCORES = 8
BC = B // NCORES          # 16 samples per core
R = T * BC                # 1600 rows (t,b) per core
KC = D // 128             # 16 contraction chunks
HC = H // 128             # 16 output-feature chunks
BETA = 0.9
NR = 256                  # row-slice width (multiple of BC)
SLICES = [(r0, min(NR, R - r0)) for r0 in range(0, R, NR)]

import os
_DEBUG_SPIKES = bool(os.environ.get("SNN_DEBUG_SPIKES"))
F32 = mybir.dt.float32
BF16 = mybir.dt.bfloat16
ALU = mybir.AluOpType
ACTF = mybir.ActivationFunctionType


def _build_nc():
    nc = bacc.Bacc("TRN2", target_bir_lowering=False)

    xT_d = nc.dram_tensor("xT", [KC, 128, R], BF16, kind="ExternalInput")
    w_d = [
        nc.dram_tensor("w1", [D, H], BF16, kind="ExternalInput"),
        nc.dram_tensor("w2", [H, H], BF16, kind="ExternalInput"),
        nc.dram_tensor("w3", [H, H], BF16, kind="ExternalInput"),
    ]
    wo_d = nc.dram_tensor("wo", [H, C], BF16, kind="ExternalInput")
    bias_d = nc.dram_tensor("biases", [128, 3 * HC], F32, kind="ExternalInput")
    bo_d = nc.dram_tensor("biaso", [C, 1], F32, kind="ExternalInput")
    out_d = nc.dram_tensor("out", [C, BC], F32, kind="ExternalOutput")

    with tile.TileContext(nc) as tc:
        with (
            tc.tile_pool(name="spool", bufs=2) as spool,
            tc.tile_pool(name="wpool", bufs=1) as wpool,
            tc.tile_pool(name="stream", bufs=3) as stream,
            tc.tile_pool(name="small", bufs=1) as small,
            tc.tile_pool(name="pspool", bufs=8, space="PSUM") as pspool,
        ):
            # Persistent big tensors
            S1 = spool.tile([128, KC * R], BF16, tag="S")
            S2 = spool.tile([128, KC * R], BF16, tag="S")
            S3 = spool.tile([128, KC * R], BF16, tag="S")  # reuses S1's slot
            w_sb = [
                wpool.tile([128, KC * H], BF16, tag="W", name=f"w{i}_sb")
                for i in range(3)
            ]
            wo_sb = small.tile([128, KC * C], BF16)

            # Small state: fp32 [128, 1024] packs mems/biases/output-layer state
            st = small.tile([128, 1152], F32)
            mem = [
                st[:, 0:256].rearrange("p (c b) -> p c b", c=KC),
                st[:, 256:512].rearrange("p (c b) -> p c b", c=KC),
                st[:, 512:768].rearrange("p (c b) -> p c b", c=KC),
            ]
            bias_sb = st[:, 768:816]            # [128, 48] = 3 layers x 16 chunks
            memo = st[:100, 816:832]            # [100, 16]
            ssum = st[:100, 832:848]
            zo = st[:100, 848:864]              # zeros (Lo t=0 s_prev)
            so_ring = st[:100, 864:896]         # [100, 32] ping-pong spikes
            bo_sb = st[:100, 896:897]           # [100, 1]
            zeros_bf = small.tile([128, 256], BF16)
            z3 = zeros_bf.rearrange("p (c b) -> p c b", c=KC)

            nc.gpsimd.memset(st[:], 0.0)
            nc.gpsimd.memset(zeros_bf[:], 0.0)
            nc.sync.dma_start(bias_sb, bias_d[:])
            nc.sync.dma_start(bo_sb, bo_d[:])
            for kc in range(KC):
                nc.sync.dma_start(
                    wo_sb[:, kc * C:(kc + 1) * C], wo_d[kc * 128:(kc + 1) * 128, :]
                )

            def dense_layer(li, rhs_of, S_out):
                """One hidden layer: matmul all row-slices + LIF scan over T."""
                w = w_sb[li]
                for kc in range(KC):
                    nc.sync.dma_start(
                        w[:, kc * H:(kc + 1) * H],
                        w_d[li][kc * 128:(kc + 1) * 128, :],
                    )
                S_out3 = S_out.rearrange("p (c r) -> p c r", c=KC)
                m3 = mem[li]
                for r0, nr in SLICES:
                    rhs = rhs_of(r0, nr)
                    cur = stream.tile([128, KC * NR], BF16, tag="stream", name="cur")
                    for hc in range(HC):
                        ps = pspool.tile([128, NR], F32, tag="ps", name="ps")
                        for kc in range(KC):
                            nc.tensor.matmul(
                                ps[:, :nr],
                                w[:, kc * H + hc * 128: kc * H + hc * 128 + 128],
                                rhs(kc),
                                start=(kc == 0),
                                stop=(kc == KC - 1),
                            )
                        nc.scalar.activation(
                            cur[:, hc * nr:(hc + 1) * nr],
                            ps[:, :nr],
                            ACTF.Identity,
                            bias=bias_sb[:, li * HC + hc: li * HC + hc + 1],
                            scale=1.0,
                        )
                    cur3 = cur[:, : KC * nr].rearrange("p (c r) -> p c r", c=KC)
                    for tl in range(nr // BC):
                        t = r0 // BC + tl
                        cur_t = cur3[:, :, tl * BC:(tl + 1) * BC]
                        s_prev = (
                            S_out3[:, :, (t - 1) * BC: t * BC] if t > 0 else z3
                        )
                        s_new = S_out3[:, :, t * BC:(t + 1) * BC]
                        # tmp = beta*mem + cur
                        nc.vector.scalar_tensor_tensor(
                            m3, m3, BETA, cur_t, ALU.mult, ALU.add
                        )
                        # spike = (tmp - 1) > s_prev   (== mem_new > 1)
                        nc.vector.scalar_tensor_tensor(
                            s_new, m3, 1.0, s_prev, ALU.subtract, ALU.is_gt
                        )
                        # mem_new = tmp - s_prev
                        nc.vector.tensor_tensor(m3, m3, s_prev, ALU.subtract)

            # ---- Layer 1: rhs streamed from HBM (x^T, host-pretransposed)
            def rhs_layer1(r0, nr):
                xin = stream.tile([128, KC * NR], BF16, tag="stream", name="xin")
                for kc in range(KC):
                    nc.sync.dma_start(
                        xin[:, kc * nr:(kc + 1) * nr], xT_d[kc][:, r0:r0 + nr]
                    )
                return lambda kc: xin[:, kc * nr:(kc + 1) * nr]

            dense_layer(0, rhs_layer1, S1)

            # ---- Layers 2, 3: rhs from previous layer's spikes in SBUF
            def rhs_from(S_in):
                S_in3 = S_in.rearrange("p (c r) -> p c r", c=KC)
                def f(r0, nr):
                    return lambda kc: S_in3[:, kc, r0:r0 + nr]
                return f

            dense_layer(1, rhs_from(S1), S2)
            dense_layer(2, rhs_from(S2), S3)

            # ---- Output layer + spike-count accumulation
            S3_3 = S3.rearrange("p (c r) -> p c r", c=KC)
            for r0, nr in SLICES:
                ps = pspool.tile([128, NR], F32, tag="ps", name="pso")
                for kc in range(KC):
                    nc.tensor.matmul(
                        ps[:100, :nr],
                        wo_sb[:, kc * C:(kc + 1) * C],
                        S3_3[:, kc, r0:r0 + nr],
                        start=(kc == 0),
                        stop=(kc == KC - 1),
                    )
                curo = stream.tile([128, NR], F32, tag="stream", name="curo")
                curo_f = curo[:100, :nr]
                nc.scalar.activation(
                    curo_f, ps[:100, :nr], ACTF.Identity,
                    bias=bo_sb, scale=1.0,
                )
                for tl in range(nr // BC):
                    t = r0 // BC + tl
                    cur_t = curo_f[:, tl * BC:(tl + 1) * BC]
                    so_prev = zo if t == 0 else so_ring[:, (1 - t % 2) * BC:(2 - t % 2) * BC]
                    so_new = so_ring[:, (t % 2) * BC:(t % 2 + 1) * BC]
                    nc.vector.scalar_tensor_tensor(
                        memo, memo, BETA, cur_t, ALU.mult, ALU.add
                    )
                    nc.vector.scalar_tensor_tensor(
                        so_new, memo, 1.0, so_prev, ALU.subtract, ALU.is_gt
                    )
                    nc.vector.tensor_tensor(memo, memo, so_prev, ALU.subtract)
                    nc.vector.tensor_tensor(ssum, ssum, so_new, ALU.add)

            nc.sync.dma_start(out_d[:], ssum)

            if _DEBUG_SPIKES:
                for nm, S in (("s1_dbg", S1), ("s2_dbg", S2), ("s3_dbg", S3)):
                    sd = nc.dram_tensor(nm, [128, KC * R], BF16, kind="ExternalOutput")
                    nc.sync.dma_start(sd[:], S[:])

    nc.compile()
    return nc


_NC_CACHE = None


def _get_nc():
    global _NC_CACHE
    if _NC_CACHE is None:
        _NC_CACHE = _build_nc()
    return _NC_CACHE


def make_in_maps(x_seq, W1, b1, W2, b2, W3, b3, Wo, bo):
    bf = ml_dtypes.bfloat16
    w1 = np.ascontiguousarray(W1.astype(bf))
    w2 = np.ascontiguousarray(W2.astype(bf))
    w3 = np.ascontiguousarray(W3.astype(bf))
    wo = np.ascontiguousarray(Wo.astype(bf))
    biases = np.concatenate(
        [b.reshape(HC, 128).T for b in (b1, b2, b3)], axis=1
    ).astype(np.float32)                       # [128, 48]
    biases = np.ascontiguousarray(biases)
    bo_a = np.ascontiguousarray(bo.reshape(C, 1).astype(np.float32))
    in_maps = []
    for c in range(NCORES):
        xs = x_seq[:, c * BC:(c + 1) * BC, :]              # [T, BC, D]
        xT = xs.transpose(2, 0, 1).reshape(KC, 128, R)     # [D,(t,b)] chunked
        in_maps.append({
            "xT": np.ascontiguousarray(xT.astype(bf)),
            "w1": w1, "w2": w2, "w3": w3, "wo": wo,
            "biases": biases, "biaso": bo_a,
        })
    return in_maps


def kernel(x_seq, W1, b1, W2, b2, W3, b3, Wo, bo):
    nc = _get_nc()
    in_maps = make_in_maps(x_seq, W1, b1, W2, b2, W3, b3, Wo, bo)
    res = run_bass_kernel_spmd(nc, in_maps, core_ids=list(range(NCORES)))
    outs = [res.results[c]["out"] for c in range(NCORES)]   # each [C, BC]
    return np.concatenate([o.T for o in outs], axis=0).astype(np.float32)



# revision 12
# speedup vs baseline: 1.1158x; 1.1158x over previous
"""DenseSNN Trainium2 kernel: 4-layer LIF SNN, T=100, B=128, D=H=2048, C=100.

Strategy
--------
Layer-unrolled phases (layer-l spikes at step t depend only on layer-(l-1)
spikes at steps <= t):

    CUR_l = S_{l-1} @ W_l + b_l   (full-size GEMM over all T*B rows)
    S_l   = LIF-scan_T(CUR_l)     (elementwise in (B,H), sequential in T)

Data-parallel over batch: 16 samples/core on 8 cores, communication-free.

On-chip layout is "transposed activations": [feature -> 16 chunks x 128
partitions, (t,b) -> free axis]. The host pre-transposes x and re-assembles
the output, so the device never transposes.

Matmuls run in fp8(e4m3) with perf_mode=DoubleRow (two 128-row fp8 matmuls
fused per instruction, K=256 per MM, ~1.5x bf16 throughput at free-dim 512).
Weights are host-prescaled by 1024 (weights ~±0.022 sit below the e4m3
min-normal 2^-6; scaling moves them into the normal range, 3% rel err), x by
16; the PSUM->SBUF activation rescales by 1/scale and adds the bias.
Spikes are 0/1 — exact in fp8.

LIF runs on the vector engine in bf16 (2x DVE mode), 3 ops/step/layer:
    m = beta*m + cur            (scalar_tensor_tensor)
    m = m - r                   (tensor_tensor; r = prev spike, skipped t=0)
    r = (m > 1)                 (tensor_scalar -> bf16 ring)
GpSimd converts the bf16 spike ring slot -> fp8 spike tensor per step, off
the DVE critical chain (2-step slack window).

The output layer (C=100) is interleaved per-slice with layer 3 so its
serial LIF chain hides under layer-3 compute instead of trailing the kernel.
"""

import os
import numpy as np
import ml_dtypes

import concourse.bass as bass
import concourse.mybir as mybir
import concourse.tile as tile
from concourse import bacc
from concourse.bass_utils import run_bass_kernel_spmd

# Problem constants (hardcoded per contract)
T, B, D, H, C = 100, 128, 2048, 2048, 100
NCORES = 8
BC = B // NCORES          # 16 samples per core
R = T * BC                # 1600 rows (t,b) per core
KC = D // 128             # 16 contraction chunks of 128
KK = KC // 2              # 8 DoubleRow chunk-pairs (K=256 per matmul)
HC = H // 128             # 16 output-feature chunks
BETA = 0.9
XSCALE = 16.0             # x pre-scale before fp8 cast
WSCALE = 1024.0           # weight pre-scale before fp8 cast
NR = 512                  # row-slice width (multiple of BC)
SLICES = [(r0, min(NR, R - r0)) for r0 in range(0, R, NR)]

_DEBUG_SPIKES = bool(os.environ.get("SNN_DEBUG_SPIKES"))
F32 = mybir.dt.float32
BF16 = mybir.dt.bfloat16
FP8 = mybir.dt.float8e4
ALU = mybir.AluOpType
ACTF = mybir.ActivationFunctionType
DR = mybir.MatmulPerfMode.DoubleRow


def _build_nc():
    nc = bacc.Bacc("TRN2", target_bir_lowering=False)

    xT_d = nc.dram_tensor("xT", [KC, 128, R], FP8, kind="ExternalInput")
    w_d = [
        nc.dram_tensor("w1", [D, H], FP8, kind="ExternalInput"),
        nc.dram_tensor("w2", [H, H], FP8, kind="ExternalInput"),
        nc.dram_tensor("w3", [H, H], FP8, kind="ExternalInput"),
    ]
    wo_d = nc.dram_tensor("wo", [H, C], FP8, kind="ExternalInput")
    bias_d = nc.dram_tensor("biases", [128, 3 * HC], F32, kind="ExternalInput")
    bo_d = nc.dram_tensor("biaso", [C, 1], F32, kind="ExternalInput")
    out_d = nc.dram_tensor("out", [C, BC], F32, kind="ExternalOutput")

    with tile.TileContext(nc) as tc:
        with (
            tc.tile_pool(name="wpool", bufs=2) as wpool,       # 2x 32KB
            tc.tile_pool(name="spool", bufs=2) as spool,       # S1,S2 25.6KB ea
            tc.tile_pool(name="s3pool", bufs=2) as s3pool,     # S3 ring 8KB ea
            tc.tile_pool(name="curpool", bufs=3) as curpool,   # 16KB ea
            tc.tile_pool(name="copool", bufs=2) as copool,     # 1KB ea
            tc.tile_pool(name="xpool", bufs=2) as xpool,       # 8KB ea
            tc.tile_pool(name="small", bufs=1) as small,
            tc.tile_pool(name="pspool", bufs=7, space="PSUM") as pspool,
        ):
            # Persistent big tensors
            S1 = spool.tile([128, KC * R], FP8, tag="S")
            S2 = spool.tile([128, KC * R], FP8, tag="S")
            w_sb = [None, None]  # rotating slots; index l % 2
            # wo padded to 128 cols/chunk: DoubleRow LDWEIGHTS needs the
            # pair-dim stride %16==0 (C=100 is not); pad cols feed unused
            # PSUM partitions 100..127.
            wo_sb = small.tile([128, KC * 128], FP8)

            # Small state
            mstate = small.tile([128, 3 * 256], BF16)       # m per layer, bf16
            rring = small.tile([128, 2 * 3 * 256], BF16)    # spike ring [parity][l]
            bias_sb = small.tile([128, 3 * HC], F32)
            ost = small.tile([128, 64], F32)   # output-layer state
            memo = ost[:C, 0:16]
            ssum = ost[:C, 16:32]
            roring = [ost[:C, 32:48], ost[:C, 48:64]]
            bo_sb = small.tile([C, 1], F32)

            def m_of(li):
                return mstate[:, li * 256:(li + 1) * 256]

            def r_of(li, t):
                off = (t % 2) * 3 * 256 + li * 256
                return rring[:, off:off + 256]

            nc.gpsimd.memset(mstate[:], 0.0)
            nc.gpsimd.memset(ost[:], 0.0)
            nc.gpsimd.memset(wo_sb[:], 0.0)

            # ---- Upfront DMAs. Descriptor-gen is ~650ns each, so split the
            # streams across engine queues: sync carries only the
            # latency-critical w1; gpsimd carries biases + w2 + wo;
            # the x stream rides the scalar queue (issued in the slice loop).
            w_sb[0] = wpool.tile([128, KC * H], FP8, tag="W", name="w_a")
            for kc in range(KC):
                nc.sync.dma_start(
                    w_sb[0][:, kc * H:(kc + 1) * H],
                    w_d[0][kc * 128:(kc + 1) * 128, :],
                )
            nc.gpsimd.dma_start(bias_sb[:], bias_d[:])
            nc.gpsimd.dma_start(bo_sb[:], bo_d[:])
            w_sb[1] = wpool.tile([128, KC * H], FP8, tag="W", name="w_b")
            for kc in range(KC):
                nc.gpsimd.dma_start(
                    w_sb[1][:, kc * H:(kc + 1) * H],
                    w_d[1][kc * 128:(kc + 1) * 128, :],
                )
            for kc in range(KC):
                nc.gpsimd.dma_start(
                    wo_sb[:, kc * 128:kc * 128 + C],
                    wo_d[kc * 128:(kc + 1) * 128, :],
                )

            S1_3 = S1.rearrange("p (c r) -> p c r", c=KC)
            S2_3 = S2.rearrange("p (c r) -> p c r", c=KC)

            def matmul_slice(w, rhs3, nr, cur, li):
                """16 output chunks x 8 DoubleRow MMs + bias/rescale to bf16."""
                w3 = w.rearrange("p (c h) -> p c h", c=KC)
                scale = 1.0 / (XSCALE * WSCALE) if li == 0 else 1.0 / WSCALE
                for hc in range(HC):
                    ps = pspool.tile([128, NR], F32, tag="ps", name="ps")
                    for kk in range(KK):
                        nc.tensor.matmul(
                            ps[:, :nr],
                            w3[:, 2 * kk:2 * kk + 2, hc * 128:(hc + 1) * 128],
                            rhs3(kk),
                            start=(kk == 0),
                            stop=(kk == KK - 1),
                            perf_mode=DR,
                        )
                    nc.scalar.activation(
                        cur[:, hc * nr:(hc + 1) * nr],
                        ps[:, :nr],
                        ACTF.Identity,
                        bias=bias_sb[:, li * HC + hc: li * HC + hc + 1],
                        scale=scale,
                    )

            def lif_slice(li, r0, nr, cur, S_dst3, s_r0):
                """LIF scan over the slice's timesteps; spikes -> bf16 ring,
                gpsimd copies ring slot -> fp8 spike tensor."""
                cur3 = cur[:, :HC * nr].rearrange("p (c r) -> p c r", c=HC)
                m = m_of(li)
                for tl in range(nr // BC):
                    t = r0 // BC + tl
                    cur_t = cur3[:, :, tl * BC:(tl + 1) * BC]
                    # m = beta*m + cur
                    nc.vector.scalar_tensor_tensor(
                        m, m, BETA, cur_t, ALU.mult, ALU.add
                    )
                    if t > 0:
                        # m -= prev spike (reset by subtraction)
                        nc.vector.tensor_tensor(m, m, r_of(li, t - 1), ALU.subtract)
                    # r = (m > 1)
                    nc.vector.tensor_scalar(r_of(li, t), m, 1.0, None, ALU.is_gt)
                    # fp8 copy for next layer's matmul (off DVE critical chain)
                    sl = (t - s_r0 // BC) * BC
                    nc.gpsimd.tensor_copy(
                        S_dst3[:, :, sl:sl + BC],
                        r_of(li, t).rearrange("p (c b) -> p c b", c=KC),
                    )

            def out_slice(s3_slot3, r0, nr):
                """Output layer for one slice: matmul + LIF + spike-count."""
                pso = pspool.tile([128, NR], F32, tag="ps", name="pso")
                for kk in range(KK):
                    nc.tensor.matmul(
                        pso[:, :nr],
                        wo_sb.rearrange("p (c h) -> p c h", c=KC)[
                            :, 2 * kk:2 * kk + 2, :],
                        s3_slot3[:, 2 * kk:2 * kk + 2, :nr],
                        start=(kk == 0),
                        stop=(kk == KK - 1),
                        perf_mode=DR,
                    )
                curo = copool.tile([128, NR], BF16, tag="curo", name="curo")
                curo_f = curo[:C, :nr]
                nc.scalar.activation(
                    curo_f, pso[:C, :nr], ACTF.Identity,
                    bias=bo_sb, scale=1.0 / WSCALE,
                )
                for tl in range(nr // BC):
                    t = r0 // BC + tl
                    cur_t = curo_f[:, tl * BC:(tl + 1) * BC]
                    nc.vector.scalar_tensor_tensor(
                        memo, memo, BETA, cur_t, ALU.mult, ALU.add
                    )
                    if t > 0:
                        nc.vector.tensor_tensor(
                            memo, memo, roring[(t - 1) % 2], ALU.subtract
                        )
                    nc.vector.tensor_scalar(
                        roring[t % 2], memo, 1.0, None, ALU.is_gt
                    )
                    nc.vector.tensor_tensor(ssum, ssum, roring[t % 2], ALU.add)

            # ---- Layer 1: rhs streamed from HBM (x^T, host-pretransposed)
            for r0, nr in SLICES:
                xin = xpool.tile([128, KC * NR], FP8, tag="x", name="xin")
                for kc in range(KC):
                    nc.scalar.dma_start(
                        xin[:, kc * nr:(kc + 1) * nr], xT_d[kc][:, r0:r0 + nr]
                    )
                xin3 = xin[:, :KC * nr].rearrange("p (c r) -> p c r", c=KC)
                cur = curpool.tile([128, HC * NR], BF16, tag="cur", name="cur")
                matmul_slice(w_sb[0], lambda kk: xin3[:, 2 * kk:2 * kk + 2, :],
                             nr, cur, 0)
                lif_slice(0, r0, nr, cur, S1_3[:, :, r0:r0 + nr], r0)

            # prefetch w3 into slot 0 (WAR on layer-1 matmuls, auto-tracked)
            w_sb[0] = wpool.tile([128, KC * H], FP8, tag="W", name="w_c")
            for kc in range(KC):
                nc.scalar.dma_start(
                    w_sb[0][:, kc * H:(kc + 1) * H],
                    w_d[2][kc * 128:(kc + 1) * 128, :],
                )

            # ---- Layer 2
            for r0, nr in SLICES:
                cur = curpool.tile([128, HC * NR], BF16, tag="cur", name="cur")
                matmul_slice(w_sb[1],
                             lambda kk: S1_3[:, 2 * kk:2 * kk + 2, r0:r0 + nr],
                             nr, cur, 1)
                lif_slice(1, r0, nr, cur, S2_3[:, :, r0:r0 + nr], r0)

            # ---- Layer 3 + output layer, interleaved per slice
            for r0, nr in SLICES:
                cur = curpool.tile([128, HC * NR], BF16, tag="cur", name="cur")
                matmul_slice(w_sb[0],
                             lambda kk: S2_3[:, 2 * kk:2 * kk + 2, r0:r0 + nr],
                             nr, cur, 2)
                s3 = s3pool.tile([128, KC * NR], FP8, tag="S3", name="s3")
                s3_3 = s3[:, :KC * nr].rearrange("p (c r) -> p c r", c=KC)
                lif_slice(2, r0, nr, cur, s3_3, r0)
                out_slice(s3_3, r0, nr)

            nc.sync.dma_start(out_d[:], ssum)

            if _DEBUG_SPIKES:
                for nm, S in (("s1_dbg", S1), ("s2_dbg", S2)):
                    sd = nc.dram_tensor(nm, [128, KC * R], FP8,
                                        kind="ExternalOutput")
                    nc.sync.dma_start(sd[:], S[:])

    nc.compile()
    return nc


_NC_CACHE = None


def _get_nc():
    global _NC_CACHE
    if _NC_CACHE is None:
        _NC_CACHE = _build_nc()
    return _NC_CACHE


def _fp8(a, scale):
    a = np.asarray(a, np.float32) * scale
    return np.ascontiguousarray(
        np.clip(a, -240.0, 240.0).astype(ml_dtypes.float8_e4m3)
    )


def make_in_maps(x_seq, W1, b1, W2, b2, W3, b3, Wo, bo):
    w1 = _fp8(W1, WSCALE)
    w2 = _fp8(W2, WSCALE)
    w3 = _fp8(W3, WSCALE)
    wo = _fp8(Wo, WSCALE)
    biases = np.concatenate(
        [np.asarray(b, np.float32).reshape(HC, 128).T for b in (b1, b2, b3)],
        axis=1,
    )
    biases = np.ascontiguousarray(biases)                 # [128, 48]
    bo_a = np.ascontiguousarray(np.asarray(bo, np.float32).reshape(C, 1))
    in_maps = []
    for c in range(NCORES):
        xs = np.asarray(x_seq[:, c * BC:(c + 1) * BC, :], np.float32)
        xT = xs.transpose(2, 0, 1).reshape(KC, 128, R)    # [D,(t,b)] chunked
        in_maps.append({
            "xT": _fp8(xT, XSCALE),
            "w1": w1, "w2": w2, "w3": w3, "wo": wo,
            "biases": biases, "biaso": bo_a,
        })
    return in_maps


def kernel(x_seq, W1, b1, W2, b2, W3, b3, Wo, bo):
    nc = _get_nc()
    in_maps = make_in_maps(x_seq, W1, b1, W2, b2, W3, b3, Wo, bo)
    res = run_bass_kernel_spmd(nc, in_maps, core_ids=list(range(NCORES)))
    outs = [res.results[c]["out"] for c in range(NCORES)]   # each [C, BC]
    return np.concatenate([o.T for o in outs], axis=0).astype(np.float32)


# revision 15
# speedup vs baseline: 1.3093x; 1.1734x over previous
"""DenseSNN Trainium2 kernel: 4-layer LIF SNN, T=100, B=128, D=H=2048, C=100.

Strategy
--------
Layer-unrolled phases (layer-l spikes at step t depend only on layer-(l-1)
spikes at steps <= t):

    CUR_l = S_{l-1} @ W_l + b_l   (full-size GEMM over all T*B rows)
    S_l   = LIF-scan_T(CUR_l)     (elementwise in (B,H), sequential in T)

Data-parallel over batch: 16 samples/core on 8 cores, communication-free.

On-chip layout is "transposed activations": [feature -> 16 chunks x 128
partitions, (t,b) -> free axis]; the host pre-transposes x and re-assembles
the output, so the device never transposes.

Matmuls run in fp8(e4m3) with perf_mode=DoubleRow (two 128-row fp8 matmuls
fused per instruction, K=256 per MM). Weights are host-prescaled by 1024
(raw weights ~±0.022 sit below the e4m3 min-normal 2^-6), x by 16; the
PSUM->SBUF activation rescales and adds the bias. Spikes are 0/1 — exact
in fp8. Row slices are 4x400 (even): at N=400 the ~190ns stream time
roughly matches the 256-col DoubleRow LDWEIGHTS, so weight loads hide; a
3x512+64 split would leave a LDWEIGHTS-bound tail.

LIF runs on the vector engine in bf16, 3 ops/step/layer:
    m = beta*m + cur     (cur written per-step-contiguous by the activation)
    m = m - r[t-1]       (reset by subtraction; skipped at t=0)
    r[t] = (m > 1)       (-> small bf16 ring, 2 x 5-step batches)
Spike fp8 conversion happens OFF the DVE critical chain as 5-step batched
casts: gpsimd for layers 1-2 (large slack), scalar for layer 3 (short
slack; gpsimd per-op overhead is ~1us and its SBUF port contends with the
DVE, so it only gets the low-rate batched work).

The output layer (C=100) is interleaved with layer 3, shifted one slice
later so the layer-3 spike casts have a full slice of slack before the
output matmuls consume them.
"""

import os
import numpy as np
import ml_dtypes

import concourse.bass as bass
import concourse.mybir as mybir
import concourse.tile as tile
from concourse import bacc
from concourse.bass_utils import run_bass_kernel_spmd

# Problem constants (hardcoded per contract)
T, B, D, H, C = 100, 128, 2048, 2048, 100
NCORES = 8
BC = B // NCORES          # 16 samples per core
R = T * BC                # 1600 rows (t,b) per core
KC = D // 128             # 16 contraction chunks of 128
KK = KC // 2              # 8 DoubleRow chunk-pairs (K=256 per matmul)
HC = H // 128             # 16 output-feature chunks
BETA = 0.9
XSCALE = 16.0             # x pre-scale before fp8 cast
WSCALE = 1024.0           # weight pre-scale before fp8 cast
NR = 400                  # row-slice width (25 steps x BC)
NS = R // NR              # 4 slices
NT = NR // BC             # 25 steps per slice
CB = 5                    # spike-cast batch (steps per fp8 cast)
SLICES = [(r0, NR) for r0 in range(0, R, NR)]

_DEBUG_SPIKES = bool(os.environ.get("SNN_DEBUG_SPIKES"))
F32 = mybir.dt.float32
BF16 = mybir.dt.bfloat16
FP8 = mybir.dt.float8e4
ALU = mybir.AluOpType
ACTF = mybir.ActivationFunctionType
DR = mybir.MatmulPerfMode.DoubleRow


def _build_nc():
    nc = bacc.Bacc("TRN2", target_bir_lowering=False)

    xT_d = nc.dram_tensor("xT", [KC, 128, R], FP8, kind="ExternalInput")
    w_d = [
        nc.dram_tensor("w1", [D, H], FP8, kind="ExternalInput"),
        nc.dram_tensor("w2", [H, H], FP8, kind="ExternalInput"),
        nc.dram_tensor("w3", [H, H], FP8, kind="ExternalInput"),
    ]
    wo_d = nc.dram_tensor("wo", [H, C], FP8, kind="ExternalInput")
    bias_d = nc.dram_tensor("biases", [128, 3 * HC], F32, kind="ExternalInput")
    bo_d = nc.dram_tensor("biaso", [C, 1], F32, kind="ExternalInput")
    out_d = nc.dram_tensor("out", [C, BC], F32, kind="ExternalOutput")

    with tile.TileContext(nc) as tc:
        with (
            tc.tile_pool(name="wpool", bufs=2) as wpool,       # 2x 32KB
            tc.tile_pool(name="spool", bufs=2) as spool,       # S1,S2 25.6KB ea
            tc.tile_pool(name="s3pool", bufs=2) as s3pool,     # 6.25KB ea
            tc.tile_pool(name="curpool", bufs=3) as curpool,   # 12.5KB ea
            tc.tile_pool(name="copool", bufs=2) as copool,     # 0.8KB ea
            tc.tile_pool(name="xpool", bufs=2) as xpool,       # 6.25KB ea
            tc.tile_pool(name="small", bufs=1) as small,
            tc.tile_pool(name="pspool", bufs=7, space="PSUM") as pspool,
        ):
            # Persistent big tensors
            S1 = spool.tile([128, KC * R], FP8, tag="S")
            S2 = spool.tile([128, KC * R], FP8, tag="S")
            w_sb = [None, None]  # rotating slots
            # wo padded to 128 cols/chunk: DoubleRow LDWEIGHTS needs the
            # pair-dim stride %16==0 (C=100 is not); pad cols feed unused
            # PSUM partitions 100..127.
            wo_sb = small.tile([128, KC * 128], FP8)

            # Small state
            mstate = small.tile([128, 3 * 256], BF16)
            # spike ring: per layer 2 batches x CB steps x 256, (c,b) order
            rring = small.tile([128, 3 * 2 * CB * 256], BF16)
            bias_sb = small.tile([128, 3 * HC], F32)
            ost = small.tile([128, 64], F32)   # output-layer state
            memo = ost[:C, 0:16]
            ssum = ost[:C, 16:32]
            roring = [ost[:C, 32:48], ost[:C, 48:64]]
            bo_sb = small.tile([C, 1], F32)

            def m_of(li):
                return mstate[:, li * 256:(li + 1) * 256]

            def r_of(li, t):
                off = (li * 2 * CB + ((t // CB) % 2) * CB + t % CB) * 256
                return rring[:, off:off + 256]

            nc.gpsimd.memset(mstate[:], 0.0)
            nc.gpsimd.memset(ost[:], 0.0)
            nc.gpsimd.memset(wo_sb[:], 0.0)

            # ---- Upfront DMAs. Descriptor-gen is ~650ns each; sync carries
            # only the latency-critical w1, gpsimd everything else.
            w_sb[0] = wpool.tile([128, KC * H], FP8, tag="W", name="w_a")
            for kc in range(KC):
                nc.sync.dma_start(
                    w_sb[0][:, kc * H:(kc + 1) * H],
                    w_d[0][kc * 128:(kc + 1) * 128, :],
                )
            nc.gpsimd.dma_start(bias_sb[:], bias_d[:])
            nc.gpsimd.dma_start(bo_sb[:], bo_d[:])
            w_sb[1] = wpool.tile([128, KC * H], FP8, tag="W", name="w_b")
            for kc in range(KC):
                nc.gpsimd.dma_start(
                    w_sb[1][:, kc * H:(kc + 1) * H],
                    w_d[1][kc * 128:(kc + 1) * 128, :],
                )
            for kc in range(KC):
                nc.gpsimd.dma_start(
                    wo_sb[:, kc * 128:kc * 128 + C],
                    wo_d[kc * 128:(kc + 1) * 128, :],
                )

            S1_3 = S1.rearrange("p (c r) -> p c r", c=KC)
            S2_3 = S2.rearrange("p (c r) -> p c r", c=KC)

            def matmul_slice(w, rhs3, nr, cur, li):
                """16 output chunks x 8 DoubleRow MMs; bias/rescale writes cur
                in per-step layout [p, t, (c b)] so LIF reads contiguously."""
                w3 = w.rearrange("p (c h) -> p c h", c=KC)
                cur3 = cur.rearrange("p (t x) -> p t x", t=NT)
                scale = 1.0 / (XSCALE * WSCALE) if li == 0 else 1.0 / WSCALE
                for hc in range(HC):
                    ps = pspool.tile([128, 512], F32, tag="ps", name="ps")
                    for kk in range(KK):
                        nc.tensor.matmul(
                            ps[:, :nr],
                            w3[:, 2 * kk:2 * kk + 2, hc * 128:(hc + 1) * 128],
                            rhs3(kk),
                            start=(kk == 0),
                            stop=(kk == KK - 1),
                            perf_mode=DR,
                        )
                    nc.scalar.activation(
                        cur3[:, :, hc * BC:(hc + 1) * BC],
                        ps[:, :nr].rearrange("p (t b) -> p t b", t=NT),
                        ACTF.Identity,
                        bias=bias_sb[:, li * HC + hc: li * HC + hc + 1],
                        scale=scale,
                    )

            def cast_batch(li, t0, S_3, s_t0):
                """fp8-convert CB steps of spikes from the bf16 ring into the
                spike tensor (matmul rhs layout [p, c, r])."""
                base = (li * 2 * CB + ((t0 // CB) % 2) * CB) * 256
                src = rring[:, base:base + CB * 256].rearrange(
                    "p (s c b) -> p s c b", s=CB, c=KC)
                w0 = (t0 - s_t0) * BC
                dst = S_3[:, :, w0:w0 + CB * BC].rearrange(
                    "p c (s b) -> p s c b", s=CB)
                if li < 2:
                    nc.gpsimd.tensor_copy(dst, src)
                else:
                    nc.scalar.copy(dst, src)

            def lif_slice(li, r0, cur, S_3, s_t0):
                m = m_of(li)
                for tl in range(NT):
                    t = r0 // BC + tl
                    nc.vector.scalar_tensor_tensor(
                        m, m, BETA, cur[:, tl * 256:(tl + 1) * 256],
                        ALU.mult, ALU.add
                    )
                    if t > 0:
                        nc.vector.tensor_tensor(m, m, r_of(li, t - 1), ALU.subtract)
                    nc.vector.tensor_scalar(r_of(li, t), m, 1.0, None, ALU.is_gt)
                    if t % CB == CB - 1:
                        cast_batch(li, t - CB + 1, S_3, s_t0)

            def out_slice(s3_3, r0, nr):
                """Output layer for one slice: matmul + LIF + spike-count."""
                pso = pspool.tile([128, 512], F32, tag="ps", name="pso")
                for kk in range(KK):
                    nc.tensor.matmul(
                        pso[:, :nr],
                        wo_sb.rearrange("p (c h) -> p c h", c=KC)[
                            :, 2 * kk:2 * kk + 2, :],
                        s3_3[:, 2 * kk:2 * kk + 2, :nr],
                        start=(kk == 0),
                        stop=(kk == KK - 1),
                        perf_mode=DR,
                    )
                curo = copool.tile([128, NR], BF16, tag="curo", name="curo")
                curo_f = curo[:C, :nr]
                nc.scalar.activation(
                    curo_f, pso[:C, :nr], ACTF.Identity,
                    bias=bo_sb, scale=1.0 / WSCALE,
                )
                for tl in range(NT):
                    t = r0 // BC + tl
                    cur_t = curo_f[:, tl * BC:(tl + 1) * BC]
                    nc.vector.scalar_tensor_tensor(
                        memo, memo, BETA, cur_t, ALU.mult, ALU.add
                    )
                    if t > 0:
                        nc.vector.tensor_tensor(
                            memo, memo, roring[(t - 1) % 2], ALU.subtract
                        )
                    nc.vector.tensor_scalar(
                        roring[t % 2], memo, 1.0, None, ALU.is_gt
                    )
                    nc.vector.tensor_tensor(ssum, ssum, roring[t % 2], ALU.add)

            # ---- Layer 1: rhs streamed from HBM (x^T, host-pretransposed).
            # x DMAs prefetch one slice ahead on the gpsimd queue.
            xin_t = [None] * NS

            def xin_fetch(j):
                xin_t[j] = xpool.tile([128, KC * NR], FP8, tag="x", name="xin")
                for kc in range(KC):
                    nc.gpsimd.dma_start(
                        xin_t[j][:, kc * NR:(kc + 1) * NR],
                        xT_d[kc][:, j * NR:(j + 1) * NR],
                    )

            xin_fetch(0)
            for j, (r0, nr) in enumerate(SLICES):
                xin3 = xin_t[j].rearrange("p (c r) -> p c r", c=KC)
                cur = curpool.tile([128, NT * 256], BF16, tag="cur", name="cur")
                matmul_slice(w_sb[0], lambda kk: xin3[:, 2 * kk:2 * kk + 2, :],
                             nr, cur, 0)
                if j + 1 < NS:
                    xin_fetch(j + 1)
                lif_slice(0, r0, cur, S1_3, 0)

            # prefetch w3 into slot 0 (WAR on layer-1 matmuls, auto-tracked)
            w_sb[0] = wpool.tile([128, KC * H], FP8, tag="W", name="w_c")
            for kc in range(KC):
                nc.gpsimd.dma_start(
                    w_sb[0][:, kc * H:(kc + 1) * H],
                    w_d[2][kc * 128:(kc + 1) * 128, :],
                )

            # ---- Layer 2
            for r0, nr in SLICES:
                cur = curpool.tile([128, NT * 256], BF16, tag="cur", name="cur")
                matmul_slice(w_sb[1],
                             lambda kk: S1_3[:, 2 * kk:2 * kk + 2, r0:r0 + nr],
                             nr, cur, 1)
                lif_slice(1, r0, cur, S2_3, 0)

            # ---- Layer 3 + output layer, interleaved one slice behind
            prev = None
            for r0, nr in SLICES:
                cur = curpool.tile([128, NT * 256], BF16, tag="cur", name="cur")
                matmul_slice(w_sb[0],
                             lambda kk: S2_3[:, 2 * kk:2 * kk + 2, r0:r0 + nr],
                             nr, cur, 2)
                if prev is not None:
                    out_slice(*prev)
                s3 = s3pool.tile([128, KC * NR], FP8, tag="S3", name="s3")
                s3_3 = s3.rearrange("p (c r) -> p c r", c=KC)
                lif_slice(2, r0, cur, s3_3, r0 // BC)
                prev = (s3_3, r0, nr)
            out_slice(*prev)

            nc.sync.dma_start(out_d[:], ssum)

            if _DEBUG_SPIKES:
                for nm, S in (("s1_dbg", S1), ("s2_dbg", S2)):
                    sd = nc.dram_tensor(nm, [128, KC * R], FP8,
                                        kind="ExternalOutput")
                    nc.sync.dma_start(sd[:], S[:])

    nc.compile()
    return nc


_NC_CACHE = None


def _get_nc():
    global _NC_CACHE
    if _NC_CACHE is None:
        _NC_CACHE = _build_nc()
    return _NC_CACHE


def _fp8(a, scale):
    a = np.asarray(a, np.float32) * scale
    return np.ascontiguousarray(
        np.clip(a, -240.0, 240.0).astype(ml_dtypes.float8_e4m3)
    )


def make_in_maps(x_seq, W1, b1, W2, b2, W3, b3, Wo, bo):
    w1 = _fp8(W1, WSCALE)
    w2 = _fp8(W2, WSCALE)
    w3 = _fp8(W3, WSCALE)
    wo = _fp8(Wo, WSCALE)
    biases = np.concatenate(
        [np.asarray(b, np.float32).reshape(HC, 128).T for b in (b1, b2, b3)],
        axis=1,
    )
    biases = np.ascontiguousarray(biases)                 # [128, 48]
    bo_a = np.ascontiguousarray(np.asarray(bo, np.float32).reshape(C, 1))
    in_maps = []
    for c in range(NCORES):
        xs = np.asarray(x_seq[:, c * BC:(c + 1) * BC, :], np.float32)
        xT = xs.transpose(2, 0, 1).reshape(KC, 128, R)    # [D,(t,b)] chunked
        in_maps.append({
            "xT": _fp8(xT, XSCALE),
            "w1": w1, "w2": w2, "w3": w3, "wo": wo,
            "biases": biases, "biaso": bo_a,
        })
    return in_maps


def kernel(x_seq, W1, b1, W2, b2, W3, b3, Wo, bo):
    nc = _get_nc()
    in_maps = make_in_maps(x_seq, W1, b1, W2, b2, W3, b3, Wo, bo)
    res = run_bass_kernel_spmd(nc, in_maps, core_ids=list(range(NCORES)))
    outs = [res.results[c]["out"] for c in range(NCORES)]   # each [C, BC]
    return np.concatenate([o.T for o in outs], axis=0).astype(np.float32)


# revision 16
# speedup vs baseline: 1.6782x; 1.2818x over previous
"""DenseSNN Trainium2 kernel: 4-layer LIF SNN, T=100, B=128, D=H=2048, C=100.

Strategy
--------
Layer-unrolled phases (layer-l spikes at step t depend only on layer-(l-1)
spikes at steps <= t):

    CUR_l = S_{l-1} @ W_l + b_l   (full-size GEMM over all T*B rows)
    S_l   = LIF-scan_T(CUR_l)     (elementwise in (B,H), sequential in T)

Data-parallel over batch: 16 samples/core on 8 cores, communication-free.

On-chip layout is "transposed activations": [feature -> 16 chunks x 128
partitions, (t,b) -> free axis]; the host pre-transposes x and re-assembles
the output, so the device never transposes.

Matmuls run in fp8(e4m3) with perf_mode=DoubleRow (two 128-row fp8 matmuls
fused per instruction, K=256 per MM). Weights are host-prescaled by 1024
(raw weights ~±0.022 sit below the e4m3 min-normal 2^-6), x by 16; the
PSUM->SBUF activation rescales and adds the bias. Spikes are 0/1 — exact
in fp8. Row slices are 4x400 (even): at N=400 the ~190ns stream time
roughly matches the 256-col DoubleRow LDWEIGHTS, so weight loads hide; a
3x512+64 split would leave a LDWEIGHTS-bound tail.

LIF runs on the vector engine in bf16, 3 ops/step/layer:
    m = beta*m + cur     (cur written per-step-contiguous by the activation)
    m = m - r[t-1]       (reset by subtraction; skipped at t=0)
    r[t] = (m > 1)       (-> small bf16 ring, 2 x 5-step batches)
Spike fp8 conversion happens OFF the DVE critical chain as 5-step batched
casts: gpsimd for layers 1-2 (large slack), scalar for layer 3 (short
slack; gpsimd per-op overhead is ~1us and its SBUF port contends with the
DVE, so it only gets the low-rate batched work).

The output layer (C=100) is interleaved with layer 3, shifted one slice
later so the layer-3 spike casts have a full slice of slack before the
output matmuls consume them.
"""

import os
import numpy as np
import ml_dtypes

import concourse.bass as bass
import concourse.mybir as mybir
import concourse.tile as tile
from concourse import bacc
from concourse.bass_utils import run_bass_kernel_spmd

# Problem constants (hardcoded per contract)
T, B, D, H, C = 100, 128, 2048, 2048, 100
NCORES = 8
BC = B // NCORES          # 16 samples per core
R = T * BC                # 1600 rows (t,b) per core
KC = D // 128             # 16 contraction chunks of 128
KK = KC // 2              # 8 DoubleRow chunk-pairs (K=256 per matmul)
HC = H // 128             # 16 output-feature chunks
BETA = 0.9
XSCALE = 16.0             # x pre-scale before fp8 cast
WSCALE = 1024.0           # weight pre-scale before fp8 cast
NR = 400                  # row-slice width (25 steps x BC)
NS = R // NR              # 4 slices
NT = NR // BC             # 25 steps per slice
CB = 5                    # spike-cast batch (steps per fp8 cast)
SLICES = [(r0, NR) for r0 in range(0, R, NR)]

_DEBUG_SPIKES = bool(os.environ.get("SNN_DEBUG_SPIKES"))
F32 = mybir.dt.float32
BF16 = mybir.dt.bfloat16
FP8 = mybir.dt.float8e4
ALU = mybir.AluOpType
ACTF = mybir.ActivationFunctionType
DR = mybir.MatmulPerfMode.DoubleRow


def _build_nc():
    nc = bacc.Bacc("TRN2", target_bir_lowering=False)

    xT_d = nc.dram_tensor("xT", [KC, 128, R], FP8, kind="ExternalInput")
    w_d = [
        nc.dram_tensor("w1", [D, H], FP8, kind="ExternalInput"),
        nc.dram_tensor("w2", [H, H], FP8, kind="ExternalInput"),
        nc.dram_tensor("w3", [H, H], FP8, kind="ExternalInput"),
    ]
    wo_d = nc.dram_tensor("wo", [H, C], FP8, kind="ExternalInput")
    bias_d = nc.dram_tensor("biases", [128, 3 * HC], F32, kind="ExternalInput")
    bo_d = nc.dram_tensor("biaso", [C, 1], F32, kind="ExternalInput")
    out_d = nc.dram_tensor("out", [C, BC], F32, kind="ExternalOutput")

    with tile.TileContext(nc) as tc:
        with (
            tc.tile_pool(name="wpool", bufs=2) as wpool,       # 2x 32KB
            tc.tile_pool(name="spool", bufs=2) as spool,       # S1,S2 25.6KB ea
            tc.tile_pool(name="s3pool", bufs=2) as s3pool,     # 6.25KB ea
            tc.tile_pool(name="curpool", bufs=3) as curpool,   # 12.5KB ea
            tc.tile_pool(name="copool", bufs=2) as copool,     # 0.8KB ea
            tc.tile_pool(name="xpool", bufs=2) as xpool,       # 6.25KB ea
            tc.tile_pool(name="small", bufs=1) as small,
            tc.tile_pool(name="pspool", bufs=7, space="PSUM") as pspool,
        ):
            # Persistent big tensors
            S1 = spool.tile([128, KC * R], FP8, tag="S")
            S2 = spool.tile([128, KC * R], FP8, tag="S")
            w_sb = [None, None]  # rotating slots
            # wo padded to 128 cols/chunk: DoubleRow LDWEIGHTS needs the
            # pair-dim stride %16==0 (C=100 is not); pad cols feed unused
            # PSUM partitions 100..127.
            wo_sb = small.tile([128, KC * 128], FP8)

            # Small state
            mstate = small.tile([128, 3 * 256], BF16)
            # spike ring: per layer 2 batches x CB steps x 256, (c,b) order
            rring = small.tile([128, 3 * 2 * CB * 256], BF16)
            bias_sb = small.tile([128, 3 * HC], F32)
            ost = small.tile([128, 64], F32)   # output-layer state
            memo = ost[:C, 0:16]
            ssum = ost[:C, 16:32]
            roring = [ost[:C, 32:48], ost[:C, 48:64]]
            bo_sb = small.tile([C, 1], F32)

            def m_of(li):
                return mstate[:, li * 256:(li + 1) * 256]

            def r_of(li, t):
                off = (li * 2 * CB + ((t // CB) % 2) * CB + t % CB) * 256
                return rring[:, off:off + 256]

            nc.gpsimd.memset(mstate[:], 0.0)
            nc.gpsimd.memset(ost[:], 0.0)
            nc.gpsimd.memset(wo_sb[:], 0.0)

            # ---- Upfront DMAs. Descriptor-gen is ~650ns each; sync carries
            # only the latency-critical w1, gpsimd everything else.
            w_sb[0] = wpool.tile([128, KC * H], FP8, tag="W", name="w_a")
            for kc in range(KC):
                nc.sync.dma_start(
                    w_sb[0][:, kc * H:(kc + 1) * H],
                    w_d[0][kc * 128:(kc + 1) * 128, :],
                )
            nc.gpsimd.dma_start(bias_sb[:], bias_d[:])
            nc.gpsimd.dma_start(bo_sb[:], bo_d[:])
            w_sb[1] = wpool.tile([128, KC * H], FP8, tag="W", name="w_b")
            for kc in range(KC):
                nc.gpsimd.dma_start(
                    w_sb[1][:, kc * H:(kc + 1) * H],
                    w_d[1][kc * 128:(kc + 1) * 128, :],
                )
            for kc in range(KC):
                nc.gpsimd.dma_start(
                    wo_sb[:, kc * 128:kc * 128 + C],
                    wo_d[kc * 128:(kc + 1) * 128, :],
                )

            S1_3 = S1.rearrange("p (c r) -> p c r", c=KC)
            S2_3 = S2.rearrange("p (c r) -> p c r", c=KC)

            def matmul_slice(w, rhs3, nr, cur, li):
                """16 output chunks x 8 DoubleRow MMs; bias/rescale writes cur
                in per-step layout [p, t, (c b)] so LIF reads contiguously."""
                w3 = w.rearrange("p (c h) -> p c h", c=KC)
                cur3 = cur.rearrange("p (t x) -> p t x", t=NT)
                scale = 1.0 / (XSCALE * WSCALE) if li == 0 else 1.0 / WSCALE
                for hc in range(HC):
                    ps = pspool.tile([128, 512], F32, tag="ps", name="ps")
                    for kk in range(KK):
                        nc.tensor.matmul(
                            ps[:, :nr],
                            w3[:, 2 * kk:2 * kk + 2, hc * 128:(hc + 1) * 128],
                            rhs3(kk),
                            start=(kk == 0),
                            stop=(kk == KK - 1),
                            perf_mode=DR,
                        )
                    nc.scalar.activation(
                        cur3[:, :, hc * BC:(hc + 1) * BC],
                        ps[:, :nr].rearrange("p (t b) -> p t b", t=NT),
                        ACTF.Identity,
                        bias=bias_sb[:, li * HC + hc: li * HC + hc + 1],
                        scale=scale,
                    )

            def cast_batch(li, t0, S_3, s_t0):
                """fp8-convert CB steps of spikes from the bf16 ring into the
                spike tensor (matmul rhs layout [p, c, r])."""
                base = (li * 2 * CB + ((t0 // CB) % 2) * CB) * 256
                src = rring[:, base:base + CB * 256].rearrange(
                    "p (s c b) -> p s c b", s=CB, c=KC)
                w0 = (t0 - s_t0) * BC
                dst = S_3[:, :, w0:w0 + CB * BC].rearrange(
                    "p c (s b) -> p s c b", s=CB)
                # scalar, not gpsimd: gpsimd's per-op overhead is ~3x and its
                # SBUF port contends with the DVE's
                nc.scalar.copy(dst, src)

            def lif_slice(li, r0, cur, S_3, s_t0):
                m = m_of(li)
                for tl in range(NT):
                    t = r0 // BC + tl
                    nc.vector.scalar_tensor_tensor(
                        m, m, BETA, cur[:, tl * 256:(tl + 1) * 256],
                        ALU.mult, ALU.add
                    )
                    if t > 0:
                        nc.vector.tensor_tensor(m, m, r_of(li, t - 1), ALU.subtract)
                    nc.vector.tensor_scalar(r_of(li, t), m, 1.0, None, ALU.is_gt)
                    if t % CB == CB - 1:
                        cast_batch(li, t - CB + 1, S_3, s_t0)

            def out_slice(s3_3, r0, nr):
                """Output layer for one slice: matmul + LIF + spike-count."""
                pso = pspool.tile([128, 512], F32, tag="ps", name="pso")
                for kk in range(KK):
                    nc.tensor.matmul(
                        pso[:, :nr],
                        wo_sb.rearrange("p (c h) -> p c h", c=KC)[
                            :, 2 * kk:2 * kk + 2, :],
                        s3_3[:, 2 * kk:2 * kk + 2, :nr],
                        start=(kk == 0),
                        stop=(kk == KK - 1),
                        perf_mode=DR,
                    )
                curo = copool.tile([128, NR], BF16, tag="curo", name="curo")
                curo_f = curo[:C, :nr]
                nc.scalar.activation(
                    curo_f, pso[:C, :nr], ACTF.Identity,
                    bias=bo_sb, scale=1.0 / WSCALE,
                )
                for tl in range(NT):
                    t = r0 // BC + tl
                    cur_t = curo_f[:, tl * BC:(tl + 1) * BC]
                    nc.vector.scalar_tensor_tensor(
                        memo, memo, BETA, cur_t, ALU.mult, ALU.add
                    )
                    if t > 0:
                        nc.vector.tensor_tensor(
                            memo, memo, roring[(t - 1) % 2], ALU.subtract
                        )
                    nc.vector.tensor_scalar(
                        roring[t % 2], memo, 1.0, None, ALU.is_gt
                    )
                    nc.vector.tensor_tensor(ssum, ssum, roring[t % 2], ALU.add)

            # ---- Layer 1: rhs streamed from HBM (x^T, host-pretransposed).
            # x DMAs prefetch one slice ahead on the gpsimd queue.
            xin_t = [None] * NS

            def xin_fetch(j):
                xin_t[j] = xpool.tile([128, KC * NR], FP8, tag="x", name="xin")
                for kc in range(KC):
                    nc.gpsimd.dma_start(
                        xin_t[j][:, kc * NR:(kc + 1) * NR],
                        xT_d[kc][:, j * NR:(j + 1) * NR],
                    )

            xin_fetch(0)
            for j, (r0, nr) in enumerate(SLICES):
                xin3 = xin_t[j].rearrange("p (c r) -> p c r", c=KC)
                cur = curpool.tile([128, NT * 256], BF16, tag="cur", name="cur")
                matmul_slice(w_sb[0], lambda kk: xin3[:, 2 * kk:2 * kk + 2, :],
                             nr, cur, 0)
                if j + 1 < NS:
                    xin_fetch(j + 1)
                lif_slice(0, r0, cur, S1_3, 0)

            # prefetch w3 into slot 0 (WAR on layer-1 matmuls, auto-tracked)
            w_sb[0] = wpool.tile([128, KC * H], FP8, tag="W", name="w_c")
            for kc in range(KC):
                nc.gpsimd.dma_start(
                    w_sb[0][:, kc * H:(kc + 1) * H],
                    w_d[2][kc * 128:(kc + 1) * 128, :],
                )

            # ---- Layer 2
            for r0, nr in SLICES:
                cur = curpool.tile([128, NT * 256], BF16, tag="cur", name="cur")
                matmul_slice(w_sb[1],
                             lambda kk: S1_3[:, 2 * kk:2 * kk + 2, r0:r0 + nr],
                             nr, cur, 1)
                lif_slice(1, r0, cur, S2_3, 0)

            # ---- Layer 3 + output layer, interleaved one slice behind
            prev = None
            for r0, nr in SLICES:
                cur = curpool.tile([128, NT * 256], BF16, tag="cur", name="cur")
                matmul_slice(w_sb[0],
                             lambda kk: S2_3[:, 2 * kk:2 * kk + 2, r0:r0 + nr],
                             nr, cur, 2)
                if prev is not None:
                    out_slice(*prev)
                s3 = s3pool.tile([128, KC * NR], FP8, tag="S3", name="s3")
                s3_3 = s3.rearrange("p (c r) -> p c r", c=KC)
                lif_slice(2, r0, cur, s3_3, r0 // BC)
                prev = (s3_3, r0, nr)
            out_slice(*prev)

            nc.sync.dma_start(out_d[:], ssum)

            if _DEBUG_SPIKES:
                for nm, S in (("s1_dbg", S1), ("s2_dbg", S2)):
                    sd = nc.dram_tensor(nm, [128, KC * R], FP8,
                                        kind="ExternalOutput")
                    nc.sync.dma_start(sd[:], S[:])

    nc.compile()
    return nc


_NC_CACHE = None


def _get_nc():
    global _NC_CACHE
    if _NC_CACHE is None:
        _NC_CACHE = _build_nc()
    return _NC_CACHE


def _fp8(a, scale):
    a = np.asarray(a, np.float32) * scale
    return np.ascontiguousarray(
        np.clip(a, -240.0, 240.0).astype(ml_dtypes.float8_e4m3)
    )


def make_in_maps(x_seq, W1, b1, W2, b2, W3, b3, Wo, bo):
    w1 = _fp8(W1, WSCALE)
    w2 = _fp8(W2, WSCALE)
    w3 = _fp8(W3, WSCALE)
    wo = _fp8(Wo, WSCALE)
    biases = np.concatenate(
        [np.asarray(b, np.float32).reshape(HC, 128).T for b in (b1, b2, b3)],
        axis=1,
    )
    biases = np.ascontiguousarray(biases)                 # [128, 48]
    bo_a = np.ascontiguousarray(np.asarray(bo, np.float32).reshape(C, 1))
    in_maps = []
    for c in range(NCORES):
        xs = np.asarray(x_seq[:, c * BC:(c + 1) * BC, :], np.float32)
        xT = xs.transpose(2, 0, 1).reshape(KC, 128, R)    # [D,(t,b)] chunked
        in_maps.append({
            "xT": _fp8(xT, XSCALE),
            "w1": w1, "w2": w2, "w3": w3, "wo": wo,
            "biases": biases, "biaso": bo_a,
        })
    return in_maps


def kernel(x_seq, W1, b1, W2, b2, W3, b3, Wo, bo):
    nc = _get_nc()
    in_maps = make_in_maps(x_seq, W1, b1, W2, b2, W3, b3, Wo, bo)
    res = run_bass_kernel_spmd(nc, in_maps, core_ids=list(range(NCORES)))
    outs = [res.results[c]["out"] for c in range(NCORES)]   # each [C, BC]
    return np.concatenate([o.T for o in outs], axis=0).astype(np.float32)
